# revision 8
# baseline (speedup 1.0000x reference)
"""Trainium2 Bass kernel for LocalWindowAttention (B=8, C=256, H=W=64, r=32).

Single-core design: the 8 NeuronCores behind the axon tunnel execute serially
(measured: an 8-core SPMD chain costs 8x a 1-core chain), so data-parallelism
buys nothing and per-core dispatch overhead is pure loss.  All 8 batch
elements run sequentially in ONE NEFF on core 0; constants load once; batches
are software-pipelined: batch b+1's x loads issue at the start of batch b's
attention loop and its 12 PSUM-consuming projection generations are drip-fed
into the shared PSUM pool (one per 4 score pairs) so the exp stream on the
Scalar engine - the bottleneck (16.8M exps/batch at 1 elem/lane/cycle) -
never stalls at batch boundaries.

Math per batch (softmax shift-invariance lets us drop the k-bias entirely):
  q4 (128=4x32 replicated groups, N) = (wq*scale replicated).T @ x + bq4
  k4 (128, N)                        = (wk replicated).T @ x
  vT chunk j (m128, C)               = x_j.T @ wv.T + bv
  S^T pair (m128, 2x512)             = k4_slice.T @ q4_slice   (4-way row-tiled)
  E = exp(S^T)  ACT PSUM->SBUF, fp8 interleaved pairs
  colsum (1, n)  ones-matmul over E;  out_u (C, n)  vT.T @ E
  out = x + (gamma/colsum) * out_u    (gamma folded into the broadcast matmul)

Dtypes: score path bf16; E and vT in fp8e4 with DoubleRow matmuls (K=256 per
instruction, 0.5 cyc/row) for the out_u and colsum accumulations; PSUM f32;
fp32 x kept in SBUF for the residual.  PSUM budget is exactly 8 banks:
score/projection pool 2x[128,1024] (4) + out accumulators 2x[128,512] (2) +
colsum/broadcast pool (2).
"""

import numpy as np
from contextlib import ExitStack

import concourse.bass as bass
import concourse.tile as tile
from concourse import bacc, mybir, bass_utils

F32 = mybir.dt.float32
BF = mybir.dt.bfloat16
FP8 = mybir.dt.float8e4
AF = mybir.ActivationFunctionType
ALU = mybir.AluOpType
DR = mybir.MatmulPerfMode.DoubleRow

B, C, HH, WW = 8, 256, 64, 64
N = HH * WW            # 4096 tokens
R = 32                 # low-rank q/k dim
NB = 512               # n-block (free dim per matmul)
NNB = N // NB          # 8
MC = 128               # m-chunk (contraction tile)
NMC = N // MC          # 32
NPAIR = NMC // 2       # 16

_cache = {}


def _build_program(nbatch=B):
    nc = bacc.Bacc("TRN2", debug=False, num_devices=1)
    x_d = nc.dram_tensor("x", (nbatch * C, N), F32, kind="ExternalInput").ap()
    wq4_d = nc.dram_tensor("wq4", (128, 2, 128), BF, kind="ExternalInput").ap()
    wk4_d = nc.dram_tensor("wk4", (128, 2, 128), BF, kind="ExternalInput").ap()
    wvt_d = nc.dram_tensor("wvt", (128, 2, 256), BF, kind="ExternalInput").ap()
    bq4_d = nc.dram_tensor("bq4", (128, 1), F32, kind="ExternalInput").ap()
    bvbc_d = nc.dram_tensor("bvbc", (128, 1024), F32, kind="ExternalInput").ap()
    onesc_d = nc.dram_tensor("onesc", (128, 2, 16), FP8, kind="ExternalInput").ap()
    grow_d = nc.dram_tensor("grow", (1, 128), BF, kind="ExternalInput").ap()
    out_d = nc.dram_tensor("out", (nbatch * C, N), F32, kind="ExternalOutput").ap()

    with tile.TileContext(nc) as tc, ExitStack() as ctx, \
         nc.allow_low_precision(reason="bf16 matmul path"):
        consts = ctx.enter_context(tc.tile_pool(name="consts", bufs=1))
        xfp = ctx.enter_context(tc.tile_pool(name="xf", bufs=2))
        xbp = ctx.enter_context(tc.tile_pool(name="xb", bufs=2))
        qkp = ctx.enter_context(tc.tile_pool(name="qk", bufs=2))
        vtp = ctx.enter_context(tc.tile_pool(name="vt", bufs=2))
        epp = ctx.enter_context(tc.tile_pool(name="ep", bufs=6))
        misc = ctx.enter_context(tc.tile_pool(name="misc", bufs=2))

        # ---- constants (loaded once) ----
        wq4_sb = consts.tile([128, 2, 128], BF, tag="wq4")
        nc.sync.dma_start(wq4_sb[:], wq4_d[:])
        wk4_sb = consts.tile([128, 2, 128], BF, tag="wk4")
        nc.sync.dma_start(wk4_sb[:], wk4_d[:])
        wvt_sb = consts.tile([128, 2, 256], BF, tag="wvt")
        nc.sync.dma_start(wvt_sb[:], wvt_d[:])
        bq4_sb = consts.tile([128, 1], F32, tag="bq4")
        nc.sync.dma_start(bq4_sb[:], bq4_d[:])
        bvbc_sb = consts.tile([128, 1024], F32, tag="bvbc")
        nc.sync.dma_start(bvbc_sb[:], bvbc_d[:])
        onesc_sb = consts.tile([128, 2, 16], FP8, tag="onesc")
        nc.sync.dma_start(onesc_sb[:], onesc_d[:])
        grow_sb = consts.tile([1, 128], BF, tag="grow")
        nc.sync.dma_start(grow_sb[:], grow_d[:])

        pbig = ctx.enter_context(
            tc.tile_pool(name="pbig", bufs=2, space=bass.MemorySpace.PSUM))
        pop = ctx.enter_context(
            tc.tile_pool(name="pop", bufs=2, space=bass.MemorySpace.PSUM))
        psm = ctx.enter_context(
            tc.tile_pool(name="psm", bufs=1, space=bass.MemorySpace.PSUM))
        pprj = ctx.enter_context(
            tc.tile_pool(name="pprj", bufs=1, space=bass.MemorySpace.PSUM))

        state = {}

        def phase_a_load(b):
            xf = {}
            xb = {}
            q4 = qkp.tile([128, N], BF, tag="q4", name=f"q4_{b}")
            k4 = qkp.tile([128, N], BF, tag="k4", name=f"k4_{b}")
            vt = vtp.tile([128, NPAIR, 2, 256], FP8, tag="vt", name=f"vt_{b}")
            state[b] = (xf, xb, q4, k4, vt)
            for qq in range(4):
                for ch in range(2):
                    t = xfp.tile([128, 1024], F32, tag=f"xf{ch}{qq}",
                                 name=f"xf_{b}_{ch}_{qq}")
                    nc.sync.dma_start(
                        t[:], x_d[b * C + ch * 128: b * C + (ch + 1) * 128,
                                  qq * 1024:(qq + 1) * 1024])
                    tb = xbp.tile([128, 1024], BF, tag=f"xb{ch}{qq}",
                                  name=f"xb_{b}_{ch}_{qq}")
                    nc.vector.tensor_copy(tb[:], t[:])
                    xf[(ch, qq)] = t
                    xb[(ch, qq)] = tb

        def phase_a_proj_step(b, step):
            """Emit one of 32 single-bank projection generations.

            Steps 0..15: q/k projections, one 512-col half-quarter each
            (step = qq*4 + is_k*2 + half).  Steps 16..31: vT pair p = step-16
            (two m-chunks, 256 c-cols each).  A dedicated 1-bank pool keeps
            these off the score-tile pool so the drip never stalls the exp
            stream.
            """
            xf, xb, q4, k4, vt = state[b]
            if step < 16:
                qq, rem = step // 4, step % 4
                is_k, half = rem // 2, rem % 2
                (w_sb, dst, bias) = ((wk4_sb, k4, None) if is_k
                                     else (wq4_sb, q4, bq4_sb))
                pq = pprj.tile([128, 512], F32, tag="pprj",
                               name=f"pq_{b}_{step}")
                for k in range(2):
                    nc.tensor.matmul(
                        pq[:],
                        w_sb[:, k, :],
                        xb[(k, qq)][:, half * 512:(half + 1) * 512],
                        start=(k == 0), stop=(k == 1))
                lo = qq * 1024 + half * 512
                if bias is not None:
                    nc.vector.tensor_scalar_add(
                        dst[:, lo:lo + 512], pq[:], bias[:])
                else:
                    nc.vector.tensor_copy(dst[:, lo:lo + 512], pq[:])
            else:  # vT pair p: chunks 2p, 2p+1
                p = step - 16
                qq = p // 4
                off = (p % 4) * 256
                pv = pprj.tile([128, 512], F32, tag="pprj",
                               name=f"pv_{b}_{p}")
                for u in range(2):  # chunk = 2p + u
                    for k in range(2):
                        nc.tensor.matmul(
                            pv[:, u * 256:(u + 1) * 256],
                            xb[(k, qq)][:, off + u * 128:off + (u + 1) * 128],
                            wvt_sb[:, k, :],
                            start=(k == 0), stop=(k == 1))
                # bias add + cast: vt[:, p, :, :] is [128,512] flat
                nc.vector.tensor_tensor(
                    vt[:, p, :, :], pv[:], bvbc_sb[:, 0:512], ALU.add)

        NSTEP = 32

        def blocks(b, nb_lo, nb_hi, pending=(), period=4):
            pending = list(pending)
            xf, xb, q4, k4, vt = state[b]
            pair_ctr = [0]
            for nb in range(nb_lo, nb_hi):
                po_t = [pop.tile([128, NB], F32, tag="po",
                                 name=f"po_{b}_{nb}_{h}") for h in range(2)]
                pc_t = psm.tile([128, NB], F32, tag="psm", name=f"pc_{b}_{nb}")
                eps = {}

                def consume_out(t, po_t=po_t, eps=eps):
                    ep = eps[t]
                    first = (t == 0)
                    last = (t == NPAIR - 1)
                    ep_r = ep.rearrange("p (o n) -> p o n", o=2)
                    for h in range(2):
                        nc.tensor.matmul(
                            po_t[h][:],
                            vt[:, t, :, h * 128:(h + 1) * 128],
                            ep_r,
                            start=first, stop=last, perf_mode=DR)

                def colsum(t, pc_t=pc_t, eps=eps):
                    # Emitted 4 pairs behind the out-MMs: the colsum bank
                    # (shared with the broadcast tile, psm bufs=1) is only
                    # released by the previous block's DVE tail; the delay
                    # keeps that release off the PE's in-order critical path.
                    ep = eps.pop(t)
                    first = (t == 0)
                    last = (t == NPAIR - 1)
                    ep_r = ep.rearrange("p (o n) -> p o n", o=2)
                    nc.tensor.matmul(
                        pc_t[0:1, :], onesc_sb[:, :, 0:1], ep_r,
                        start=first, stop=last, perf_mode=DR)

                for t in range(NPAIR):
                    ps = pbig.tile([128, 1024], F32, tag="pbig",
                                   name=f"ps_{b}_{nb}_{t}")
                    for o in range(2):
                        j = 2 * t + o
                        g = j % 4
                        nc.tensor.matmul(
                            ps[:, o * NB:(o + 1) * NB],
                            k4[32 * g:32 * (g + 1), j * 128:(j + 1) * 128],
                            q4[32 * g:32 * (g + 1), nb * NB:(nb + 1) * NB],
                            start=True, stop=True,
                            tile_position=(32 * g, 0))
                    ep = epp.tile([128, 1024], FP8, tag="ep",
                                  name=f"ep_{b}_{nb}_{t}")
                    nc.scalar.activation(ep[:], ps[:], AF.Exp)
                    eps[t] = ep
                    if t >= 1:
                        consume_out(t - 1)
                    if t >= 4:
                        colsum(t - 4)
                    pair_ctr[0] += 1
                    if pending and pair_ctr[0] % period == 0:
                        nb_, s_ = pending.pop(0)
                        phase_a_proj_step(nb_, s_)
                consume_out(NPAIR - 1)
                for t in range(NPAIR - 4, NPAIR):
                    colsum(t)

                # ---- tail: normalize, residual, store ----
                recip = misc.tile([1, NB], BF, tag="recip", name=f"rc_{b}_{nb}")
                nc.vector.reciprocal(recip[:], pc_t[0:1, :])
                bc = psm.tile([128, NB], F32, tag="psm", name=f"bc_{b}_{nb}")
                nc.tensor.matmul(bc[:], grow_sb[:], recip[:],
                                 start=True, stop=True)
                bc_sb = misc.tile([128, NB], F32, tag="bcs", name=f"bcs_{b}_{nb}")
                nc.vector.tensor_copy(bc_sb[:], bc[:])
                for h in range(2):
                    tmp = misc.tile([128, NB], F32, tag="tmp",
                                    name=f"tmp_{b}_{nb}_{h}")
                    nc.vector.tensor_mul(tmp[:], po_t[h][:], bc_sb[:])
                    ot = misc.tile([128, NB], F32, tag="ot",
                                   name=f"ot_{b}_{nb}_{h}")
                    nc.vector.tensor_tensor(
                        ot[:], tmp[:],
                        xf[(h, nb // 2)][:, (nb % 2) * NB:(nb % 2 + 1) * NB],
                        ALU.add)
                    nc.sync.dma_start(
                        out_d[b * C + h * 128: b * C + (h + 1) * 128,
                              nb * NB:(nb + 1) * NB], ot[:])
            if nb_hi == NNB:
                del state[b]

        # Pipeline batches: batch b+1's x loads are issued at the start of
        # batch b's block loop, and its 12 PSUM-consuming projection
        # generations are drip-fed into the shared pool (one per 4 score
        # pairs) so the pool FIFO never stalls the exp stream.
        phase_a_load(0)
        for s in range(NSTEP):
            phase_a_proj_step(0, s)
        for b in range(nbatch):
            pending = []
            if b + 1 < nbatch:
                phase_a_load(b + 1)
                pending += [(b + 1, s) for s in range(NSTEP)]
            blocks(b, 0, NNB, pending, period=2)

    nc.compile()
    return nc


def _make_in_map(inputs, nbatch=B):
    import ml_dtypes
    bf = ml_dtypes.bfloat16
    x = np.asarray(inputs["x"], dtype=np.float32)
    wq = np.asarray(inputs["wq"], dtype=np.float32)
    bq = np.asarray(inputs["bq"], dtype=np.float32)
    wk = np.asarray(inputs["wk"], dtype=np.float32)
    wv = np.asarray(inputs["wv"], dtype=np.float32)
    bv = np.asarray(inputs["bv"], dtype=np.float32)
    gamma = float(np.asarray(inputs["gamma"]).reshape(-1)[0])

    scale = float(R) ** -0.5
    # wq4[c, k, g*32+r] = wq[r, 128k+c]*scale  (4x replicated along m)
    wq4 = np.tile(wq.T * scale, (1, 4)).reshape(2, 128, 128).transpose(1, 0, 2)
    wk4 = np.tile(wk.T, (1, 4)).reshape(2, 128, 128).transpose(1, 0, 2)
    # wvt[c, k, d] = wv[d, 128k+c]
    wvt = wv.T.reshape(2, 128, 256).transpose(1, 0, 2)
    return {
        "x": np.ascontiguousarray(x.reshape(nbatch * C, N)),
        "wq4": np.ascontiguousarray(wq4).astype(bf),
        "wk4": np.ascontiguousarray(wk4).astype(bf),
        "wvt": np.ascontiguousarray(wvt).astype(bf),
        "bq4": np.ascontiguousarray(np.tile(bq * scale, 4).reshape(128, 1)),
        "bvbc": np.ascontiguousarray(
            np.broadcast_to(np.tile(bv, 4)[None, :], (128, 1024))).astype(
                np.float32),
        "onesc": np.ones((128, 2, 16), dtype=ml_dtypes.float8_e4m3),
        "grow": np.full((1, 128), gamma, dtype=bf),
    }


def kernel(**inputs) -> np.ndarray:
    if "prog" not in _cache:
        _cache["prog"] = _build_program()
    nc = _cache["prog"]
    in_map = _make_in_map(inputs)
    res = bass_utils.run_bass_kernel_spmd(nc, [in_map], core_ids=[0])
    out = res.results[0]["out"].reshape(B, C, HH, WW)
    return out.astype(np.float32)


# revision 10
# speedup vs baseline: 1.2723x; 1.2723x over previous
"""Trainium2 Bass kernel for LocalWindowAttention (B=8, C=256, H=W=64, r=32).

Single-core design: the 8 NeuronCores behind the axon tunnel execute serially
(measured: an 8-core SPMD chain costs 8x a 1-core chain), so data-parallelism
buys nothing and per-core dispatch overhead is pure loss.  All 8 batch
elements run sequentially in ONE NEFF on core 0; constants load once; batches
are software-pipelined: batch b+1's x loads issue at the start of batch b's
attention loop and its 32 single-bank projection generations are drip-fed
(one per 2 score pairs) through a DEDICATED 1-bank PSUM pool, so the exp
stream on the Scalar engine - the bottleneck (16.8M exps/batch at
1 elem/lane/cycle, 95.6% busy in the cost model) - never stalls: the drip
shares only PE/DVE bandwidth, never the score-tile pool.  The colsum
matmuls trail the out-matmuls by 4 pairs so their bank (shared with the
reciprocal-broadcast tile) is released by the previous block's DVE tail
off the PE's in-order critical path.

Math per batch (softmax shift-invariance lets us drop the k-bias entirely):
  q4 (128=4x32 replicated groups, N) = (wq*scale replicated).T @ x + bq4
  k4 (128, N)                        = (wk replicated).T @ x
  vT chunk j (m128, C)               = x_j.T @ wv.T + bv
  S^T pair (m128, 2x512)             = k4_slice.T @ q4_slice   (4-way row-tiled)
  E = exp(S^T)  ACT PSUM->SBUF, fp8 interleaved pairs
  colsum (1, n)  ones-matmul over E;  out_u (C, n)  vT.T @ E
  out = x + (gamma/colsum) * out_u    (gamma folded into the broadcast matmul)

Dtypes: score path bf16; E and vT in fp8e4 with DoubleRow matmuls (K=256 per
instruction, 0.5 cyc/row) for the out_u and colsum accumulations; PSUM f32;
fp32 x kept in SBUF for the residual.  PSUM budget is exactly 8 banks:
score pool 2x[128,1024] (4) + out accumulators 2x[128,512] (2) +
colsum/broadcast pool (1) + projection pool (1).
"""

import numpy as np
from contextlib import ExitStack

import concourse.bass as bass
import concourse.tile as tile
from concourse import bacc, mybir, bass_utils

F32 = mybir.dt.float32
BF = mybir.dt.bfloat16
FP8 = mybir.dt.float8e4
AF = mybir.ActivationFunctionType
ALU = mybir.AluOpType
DR = mybir.MatmulPerfMode.DoubleRow

B, C, HH, WW = 8, 256, 64, 64
N = HH * WW            # 4096 tokens
R = 32                 # low-rank q/k dim
NB = 512               # n-block (free dim per matmul)
NNB = N // NB          # 8
MC = 128               # m-chunk (contraction tile)
NMC = N // MC          # 32
NPAIR = NMC // 2       # 16

_cache = {}


def _build_program(nbatch=B):
    nc = bacc.Bacc("TRN2", debug=False, num_devices=1)
    x_d = nc.dram_tensor("x", (nbatch * C, N), F32, kind="ExternalInput").ap()
    wq4_d = nc.dram_tensor("wq4", (128, 2, 128), BF, kind="ExternalInput").ap()
    wk4_d = nc.dram_tensor("wk4", (128, 2, 128), BF, kind="ExternalInput").ap()
    wvt_d = nc.dram_tensor("wvt", (128, 2, 256), BF, kind="ExternalInput").ap()
    bq4_d = nc.dram_tensor("bq4", (128, 1), F32, kind="ExternalInput").ap()
    bvbc_d = nc.dram_tensor("bvbc", (128, 1024), F32, kind="ExternalInput").ap()
    onesc_d = nc.dram_tensor("onesc", (128, 2, 16), FP8, kind="ExternalInput").ap()
    grow_d = nc.dram_tensor("grow", (1, 128), BF, kind="ExternalInput").ap()
    out_d = nc.dram_tensor("out", (nbatch * C, N), F32, kind="ExternalOutput").ap()

    with tile.TileContext(nc) as tc, ExitStack() as ctx, \
         nc.allow_low_precision(reason="bf16 matmul path"):
        consts = ctx.enter_context(tc.tile_pool(name="consts", bufs=1))
        xfp = ctx.enter_context(tc.tile_pool(name="xf", bufs=2))
        xbp = ctx.enter_context(tc.tile_pool(name="xb", bufs=2))
        qkp = ctx.enter_context(tc.tile_pool(name="qk", bufs=2))
        vtp = ctx.enter_context(tc.tile_pool(name="vt", bufs=2))
        epp = ctx.enter_context(tc.tile_pool(name="ep", bufs=6))
        misc = ctx.enter_context(tc.tile_pool(name="misc", bufs=2))

        # ---- constants (loaded once) ----
        wq4_sb = consts.tile([128, 2, 128], BF, tag="wq4")
        nc.sync.dma_start(wq4_sb[:], wq4_d[:])
        wk4_sb = consts.tile([128, 2, 128], BF, tag="wk4")
        nc.sync.dma_start(wk4_sb[:], wk4_d[:])
        wvt_sb = consts.tile([128, 2, 256], BF, tag="wvt")
        nc.sync.dma_start(wvt_sb[:], wvt_d[:])
        bq4_sb = consts.tile([128, 1], F32, tag="bq4")
        nc.sync.dma_start(bq4_sb[:], bq4_d[:])
        bvbc_sb = consts.tile([128, 1024], F32, tag="bvbc")
        nc.sync.dma_start(bvbc_sb[:], bvbc_d[:])
        onesc_sb = consts.tile([128, 2, 16], FP8, tag="onesc")
        nc.sync.dma_start(onesc_sb[:], onesc_d[:])
        grow_sb = consts.tile([1, 128], BF, tag="grow")
        nc.sync.dma_start(grow_sb[:], grow_d[:])

        pbig = ctx.enter_context(
            tc.tile_pool(name="pbig", bufs=2, space=bass.MemorySpace.PSUM))
        pop = ctx.enter_context(
            tc.tile_pool(name="pop", bufs=2, space=bass.MemorySpace.PSUM))
        psm = ctx.enter_context(
            tc.tile_pool(name="psm", bufs=1, space=bass.MemorySpace.PSUM))
        pprj = ctx.enter_context(
            tc.tile_pool(name="pprj", bufs=1, space=bass.MemorySpace.PSUM))

        state = {}

        def phase_a_load(b):
            xf = {}
            xb = {}
            q4 = qkp.tile([128, N], BF, tag="q4", name=f"q4_{b}")
            k4 = qkp.tile([128, N], BF, tag="k4", name=f"k4_{b}")
            vt = vtp.tile([128, NPAIR, 2, 256], FP8, tag="vt", name=f"vt_{b}")
            state[b] = (xf, xb, q4, k4, vt)
            for qq in range(4):
                for ch in range(2):
                    t = xfp.tile([128, 1024], F32, tag=f"xf{ch}{qq}",
                                 name=f"xf_{b}_{ch}_{qq}")
                    nc.sync.dma_start(
                        t[:], x_d[b * C + ch * 128: b * C + (ch + 1) * 128,
                                  qq * 1024:(qq + 1) * 1024])
                    tb = xbp.tile([128, 1024], BF, tag=f"xb{ch}{qq}",
                                  name=f"xb_{b}_{ch}_{qq}")
                    nc.vector.tensor_copy(tb[:], t[:])
                    xf[(ch, qq)] = t
                    xb[(ch, qq)] = tb

        def phase_a_proj_step(b, step):
            """Emit one of 32 single-bank projection generations.

            Steps 0..15: q/k projections, one 512-col half-quarter each
            (step = qq*4 + is_k*2 + half).  Steps 16..31: vT pair p = step-16
            (two m-chunks, 256 c-cols each).  A dedicated 1-bank pool keeps
            these off the score-tile pool so the drip never stalls the exp
            stream.
            """
            xf, xb, q4, k4, vt = state[b]
            if step < 16:
                qq, rem = step // 4, step % 4
                is_k, half = rem // 2, rem % 2
                (w_sb, dst, bias) = ((wk4_sb, k4, None) if is_k
                                     else (wq4_sb, q4, bq4_sb))
                pq = pprj.tile([128, 512], F32, tag="pprj",
                               name=f"pq_{b}_{step}")
                for k in range(2):
                    nc.tensor.matmul(
                        pq[:],
                        w_sb[:, k, :],
                        xb[(k, qq)][:, half * 512:(half + 1) * 512],
                        start=(k == 0), stop=(k == 1))
                lo = qq * 1024 + half * 512
                if bias is not None:
                    nc.vector.tensor_scalar_add(
                        dst[:, lo:lo + 512], pq[:], bias[:])
                else:
                    nc.vector.tensor_copy(dst[:, lo:lo + 512], pq[:])
            else:  # vT pair p: chunks 2p, 2p+1
                p = step - 16
                qq = p // 4
                off = (p % 4) * 256
                pv = pprj.tile([128, 512], F32, tag="pprj",
                               name=f"pv_{b}_{p}")
                for u in range(2):  # chunk = 2p + u
                    for k in range(2):
                        nc.tensor.matmul(
                            pv[:, u * 256:(u + 1) * 256],
                            xb[(k, qq)][:, off + u * 128:off + (u + 1) * 128],
                            wvt_sb[:, k, :],
                            start=(k == 0), stop=(k == 1))
                # bias add + cast: vt[:, p, :, :] is [128,512] flat
                nc.vector.tensor_tensor(
                    vt[:, p, :, :], pv[:], bvbc_sb[:, 0:512], ALU.add)

        NSTEP = 32

        def blocks(b, nb_lo, nb_hi, pending=(), period=4):
            pending = list(pending)
            xf, xb, q4, k4, vt = state[b]
            pair_ctr = [0]
            for nb in range(nb_lo, nb_hi):
                po_t = [pop.tile([128, NB], F32, tag="po",
                                 name=f"po_{b}_{nb}_{h}") for h in range(2)]
                pc_t = psm.tile([128, NB], F32, tag="psm", name=f"pc_{b}_{nb}")
                eps = {}

                def consume_out(t, po_t=po_t, eps=eps):
                    ep = eps[t]
                    first = (t == 0)
                    last = (t == NPAIR - 1)
                    ep_r = ep.rearrange("p (o n) -> p o n", o=2)
                    for h in range(2):
                        nc.tensor.matmul(
                            po_t[h][:],
                            vt[:, t, :, h * 128:(h + 1) * 128],
                            ep_r,
                            start=first, stop=last, perf_mode=DR)

                def colsum(t, pc_t=pc_t, eps=eps):
                    # Emitted 4 pairs behind the out-MMs: the colsum bank
                    # (shared with the broadcast tile, psm bufs=1) is only
                    # released by the previous block's DVE tail; the delay
                    # keeps that release off the PE's in-order critical path.
                    ep = eps.pop(t)
                    first = (t == 0)
                    last = (t == NPAIR - 1)
                    ep_r = ep.rearrange("p (o n) -> p o n", o=2)
                    nc.tensor.matmul(
                        pc_t[0:1, :], onesc_sb[:, :, 0:1], ep_r,
                        start=first, stop=last, perf_mode=DR)

                for t in range(NPAIR):
                    ps = pbig.tile([128, 1024], F32, tag="pbig",
                                   name=f"ps_{b}_{nb}_{t}")
                    for o in range(2):
                        j = 2 * t + o
                        g = j % 4
                        nc.tensor.matmul(
                            ps[:, o * NB:(o + 1) * NB],
                            k4[32 * g:32 * (g + 1), j * 128:(j + 1) * 128],
                            q4[32 * g:32 * (g + 1), nb * NB:(nb + 1) * NB],
                            start=True, stop=True,
                            tile_position=(32 * g, 0))
                    ep = epp.tile([128, 1024], FP8, tag="ep",
                                  name=f"ep_{b}_{nb}_{t}")
                    nc.scalar.activation(ep[:], ps[:], AF.Exp)
                    eps[t] = ep
                    if t >= 1:
                        consume_out(t - 1)
                    if t >= 4:
                        colsum(t - 4)
                    pair_ctr[0] += 1
                    if pending and pair_ctr[0] % period == 0:
                        nb_, s_ = pending.pop(0)
                        phase_a_proj_step(nb_, s_)
                consume_out(NPAIR - 1)
                for t in range(NPAIR - 4, NPAIR):
                    colsum(t)

                # ---- tail: normalize, residual, store ----
                recip = misc.tile([1, NB], BF, tag="recip", name=f"rc_{b}_{nb}")
                nc.vector.reciprocal(recip[:], pc_t[0:1, :])
                bc = psm.tile([128, NB], F32, tag="psm", name=f"bc_{b}_{nb}")
                nc.tensor.matmul(bc[:], grow_sb[:], recip[:],
                                 start=True, stop=True)
                bc_sb = misc.tile([128, NB], F32, tag="bcs", name=f"bcs_{b}_{nb}")
                nc.vector.tensor_copy(bc_sb[:], bc[:])
                for h in range(2):
                    tmp = misc.tile([128, NB], F32, tag="tmp",
                                    name=f"tmp_{b}_{nb}_{h}")
                    nc.vector.tensor_mul(tmp[:], po_t[h][:], bc_sb[:])
                    ot = misc.tile([128, NB], F32, tag="ot",
                                   name=f"ot_{b}_{nb}_{h}")
                    nc.vector.tensor_tensor(
                        ot[:], tmp[:],
                        xf[(h, nb // 2)][:, (nb % 2) * NB:(nb % 2 + 1) * NB],
                        ALU.add)
                    nc.sync.dma_start(
                        out_d[b * C + h * 128: b * C + (h + 1) * 128,
                              nb * NB:(nb + 1) * NB], ot[:])
            if nb_hi == NNB:
                del state[b]

        # Pipeline batches: batch b+1's x loads are issued at the start of
        # batch b's block loop, and its 12 PSUM-consuming projection
        # generations are drip-fed into the shared pool (one per 4 score
        # pairs) so the pool FIFO never stalls the exp stream.
        phase_a_load(0)
        for s in range(NSTEP):
            phase_a_proj_step(0, s)
        for b in range(nbatch):
            pending = []
            if b + 1 < nbatch:
                phase_a_load(b + 1)
                pending += [(b + 1, s) for s in range(NSTEP)]
            blocks(b, 0, NNB, pending, period=2)

    nc.compile()
    return nc


def _make_in_map(inputs, nbatch=B):
    import ml_dtypes
    bf = ml_dtypes.bfloat16
    x = np.asarray(inputs["x"], dtype=np.float32)
    wq = np.asarray(inputs["wq"], dtype=np.float32)
    bq = np.asarray(inputs["bq"], dtype=np.float32)
    wk = np.asarray(inputs["wk"], dtype=np.float32)
    wv = np.asarray(inputs["wv"], dtype=np.float32)
    bv = np.asarray(inputs["bv"], dtype=np.float32)
    gamma = float(np.asarray(inputs["gamma"]).reshape(-1)[0])

    scale = float(R) ** -0.5
    # wq4[c, k, g*32+r] = wq[r, 128k+c]*scale  (4x replicated along m)
    wq4 = np.tile(wq.T * scale, (1, 4)).reshape(2, 128, 128).transpose(1, 0, 2)
    wk4 = np.tile(wk.T, (1, 4)).reshape(2, 128, 128).transpose(1, 0, 2)
    # wvt[c, k, d] = wv[d, 128k+c]
    wvt = wv.T.reshape(2, 128, 256).transpose(1, 0, 2)
    return {
        "x": np.ascontiguousarray(x.reshape(nbatch * C, N)),
        "wq4": np.ascontiguousarray(wq4).astype(bf),
        "wk4": np.ascontiguousarray(wk4).astype(bf),
        "wvt": np.ascontiguousarray(wvt).astype(bf),
        "bq4": np.ascontiguousarray(np.tile(bq * scale, 4).reshape(128, 1)),
        "bvbc": np.ascontiguousarray(
            np.broadcast_to(np.tile(bv, 4)[None, :], (128, 1024))).astype(
                np.float32),
        "onesc": np.ones((128, 2, 16), dtype=ml_dtypes.float8_e4m3),
        "grow": np.full((1, 128), gamma, dtype=bf),
    }


def kernel(**inputs) -> np.ndarray:
    if "prog" not in _cache:
        _cache["prog"] = _build_program()
    nc = _cache["prog"]
    in_map = _make_in_map(inputs)
    res = bass_utils.run_bass_kernel_spmd(nc, [in_map], core_ids=[0])
    out = res.results[0]["out"].reshape(B, C, HH, WW)
    return out.astype(np.float32)
